# revision 17
# baseline (speedup 1.0000x reference)
"""BCRNN layer (bidirectional convolutional RNN) on 8 Trainium2 NeuronCores.

Problem: nb=1, nc=2, nt=12, nx=160, ny=160, hid=64, K=3, reflect padding,
complex conv decomposed into real convs, modReLU activation, forward +
backward temporal scans, output = sum of the two scans stacked (r, i).

Sharding: spatial rows (nx) split 8 ways (20 rows/core); temporal scans run
locally.  Halo exchange runs every SECOND step (width-2 halos): step A
computes the 20 owned rows, an AllGather + per-core indirect gather then
extends the state to +-2 rows, step B computes 22 rows (one redundant row
each side), and a small local DRAM bounce re-applies the global-edge
reflect rows so no second collective is needed.  All per-core routing
(neighbor vs reflect sources) lives in index input tensors, keeping the
single SPMD program uniform across cores.

Compute per conv: 9 shifted fp16 matmuls (K=128 = stacked real/imag
channels, M=128 = stacked real/imag outputs) accumulated in PSUM, plus one
identity matmul that accumulates pre[t] into the same psum (z never leaves
PSUM).  All matmul rhs operands are single CONTIGUOUS runs over the
padded-row layout (row stride == row length 162); psum rows are 162 wide
and the junk edge columns are overwritten by per-group reflect fixups.
i2h conv is one K=36 matmul over a host-built im2col.  modReLU:
z2 = (z/64)^2 in fp16, one 0/1-matrix matmul pairs partition halves,
rs = AbsRecipSqrt(4096*m2 + 1e-6) = 1/|z|, h = max(0, 1 + b*rs) * z.
"""
import os

import numpy as np

import concourse.bass as bass
import concourse.bacc as bacc
from concourse.bass import _add_dep_helper
import concourse.tile as tile
import concourse.mybir as mybir
from concourse.bass_utils import run_bass_kernel_spmd

P = 128
NC_CORES = 8
NT = 12
NX, NY = 160, 160
HID = 64
ROWS = NX // NC_CORES      # 20 owned rows per core
TR = ROWS + 4              # 24 tile rows (2-row halo each side)
PRE_R = ROWS + 2           # 22 pre rows (one redundant row each side)
YP = NY + 2                # 162 padded cols
FS = TR * YP + 2           # flat h/iter tile size (+1 guard col each side)
NSLOT = 12                 # AG slots: 6 per direction
RSLOT = 8                  # local refresh slots: 4 per direction

f32 = mybir.dt.float32
f16 = mybir.dt.float16
i32 = mybir.dt.int32
AF = mybir.ActivationFunctionType
ALU = mybir.AluOpType

# scan groups as (first output tile row q0, nrows); interior first,
# halo/refresh-dependent groups last
A_EARLY = [(10, 3), (7, 3), (13, 3), (4, 3), (16, 2), (18, 2)]
A_LATE = [(2, 2), (20, 2)]
B_EARLY = [(9, 3), (12, 3), (6, 3), (15, 3), (3, 3), (18, 3)]
B_LATE = [(1, 2), (21, 2)]
# pre groups over the 22 pre rows (pre row p -> iter tile row p+1)
PRE_GROUPS = [(0, 3), (3, 3), (6, 3), (9, 3), (12, 3), (15, 3), (18, 3), (21, 1)]
AG_ROWS = [2, 3, 4, 19, 20, 21]    # contributed h_A tile rows per direction
RF_ROWS = [1, 3, 20, 22]           # contributed h_B tile rows per direction
GN = 3 * YP

_CACHED = {}
TRACE = False
LAST = {}


def _row(q):
    """flat offset of (tile row q, padded col 0) in an FS-sized tile"""
    return 1 + q * YP


def _build():
    if "nc" in _CACHED:
        return _CACHED["nc"]
    nc = bacc.Bacc(None, target_bir_lowering=False, debug=False,
                   num_devices=NC_CORES)

    # ---- I/O ----
    iter_il = nc.dram_tensor("iter_il", [NT, P, TR * YP], f16, kind="ExternalInput")
    im2col = nc.dram_tensor("im2col", [NT, 36, PRE_R * YP], f16, kind="ExternalInput")
    wh2h = nc.dram_tensor("wh2h", [9, P, P], f16, kind="ExternalInput")
    wih = nc.dram_tensor("wih", [9, P, P], f16, kind="ExternalInput")
    wi2h = nc.dram_tensor("wi2h", [36, P], f16, kind="ExternalInput")
    pmat = nc.dram_tensor("pmat", [P, P], f16, kind="ExternalInput")
    imat = nc.dram_tensor("imat", [P, P], f16, kind="ExternalInput")
    b_pre = nc.dram_tensor("b_pre", [P, 1], f32, kind="ExternalInput")
    b_mod = nc.dram_tensor("b_mod", [P, 1], f32, kind="ExternalInput")
    hidx = nc.dram_tensor("hidx", [P, 8], i32, kind="ExternalInput")
    ridx = nc.dram_tensor("ridx", [P, 4], i32, kind="ExternalInput")
    out = nc.dram_tensor("out", [NT, P, ROWS, NY], f32, kind="ExternalOutput")

    # ---- internal DRAM ----
    predr = nc.dram_tensor("predr", [NT, P, PRE_R * YP], f16)
    sav = nc.dram_tensor("sav", [NT, P, ROWS, NY], f16)

    with tile.TileContext(nc) as tc:
        with (
            tc.tile_pool(name="wp", bufs=1) as wp,
            tc.tile_pool(name="dram", bufs=2, space="DRAM") as dram,
            tc.tile_pool(name="cps", bufs=2, space="PSUM") as cps,
            tc.tile_pool(name="cpb", bufs=1, space="PSUM") as cpb,
            tc.tile_pool(name="mps", bufs=2, space="PSUM") as mps,
            tc.tile_pool(name="mpb", bufs=1, space="PSUM") as mpb,
        ):
            # weights / constants
            wh = [wp.tile([P, P], f16, tag=f"wh{k}", name=f"wh{k}") for k in range(9)]
            wi = [wp.tile([P, P], f16, tag=f"wi{k}", name=f"wi{k}") for k in range(9)]
            for k in range(9):
                nc.sync.dma_start(out=wh[k][:], in_=wh2h[k])
                nc.sync.dma_start(out=wi[k][:], in_=wih[k])
            w36 = wp.tile([36, P], f16, tag="w36")
            nc.sync.dma_start(out=w36[:], in_=wi2h[:])
            pm = wp.tile([P, P], f16, tag="pm")
            nc.sync.dma_start(out=pm[:], in_=pmat[:])
            im = wp.tile([P, P], f16, tag="im")
            nc.sync.dma_start(out=im[:], in_=imat[:])
            bp = wp.tile([P, 1], f32, tag="bp")
            nc.sync.dma_start(out=bp[:], in_=b_pre[:])
            bm = wp.tile([P, 1], f32, tag="bm")
            nc.sync.dma_start(out=bm[:], in_=b_mod[:])
            hix = wp.tile([P, 8], i32, tag="hix")
            nc.sync.dma_start(out=hix[:], in_=hidx[:])
            rix = wp.tile([P, 4], i32, tag="rix")
            nc.sync.dma_start(out=rix[:], in_=ridx[:])
            epst = wp.tile([P, 1], f32, tag="epst")
            nc.vector.memset(epst[:], 1e-6)

            # ---------------- phase 1: pre[t] = ih(iter) + i2h(input) + bias
            # pre covers 22 rows (one redundant row each side of owned)
            with (
                tc.tile_pool(name="itp", bufs=4) as itp,
                tc.tile_pool(name="icp", bufs=4) as icp,
                tc.tile_pool(name="prs", bufs=3) as prs,
            ):
                # two timesteps share every tap's LDWEIGHTS so the weight
                # path stays hidden behind the matmuls (same trick as the
                # scan's f/b interleave)
                for t in range(0, NT, 2):
                    its, ics, stages = [], [], []
                    for dt_ in (0, 1):
                        it = itp.tile([P, FS], f16, tag="it", name="it")
                        nc.sync.dma_start(out=it[:, 1:1 + TR * YP],
                                          in_=iter_il[t + dt_])
                        ic = icp.tile([36, PRE_R * YP], f16, tag="ic", name="ic")
                        nc.sync.dma_start(out=ic[:], in_=im2col[t + dt_])
                        its.append(it)
                        ics.append(ic)
                        stages.append(prs.tile([P, PRE_R * YP], f16,
                                               tag="stage", name="stage"))
                    for p0, nr in PRE_GROUPS:
                        n = nr * YP
                        pss = [cps.tile([P, GN], f32, tag="cvf", name="cv"),
                               cps.tile([P, GN], f32, tag="cvb", name="cv")]
                        for tap in range(9):
                            dxi, dyi = divmod(tap, 3)
                            base = (p0 + dxi) * YP + dyi
                            for dt_ in (0, 1):
                                nc.tensor.matmul(
                                    out=pss[dt_][:, :n],
                                    lhsT=wi[tap][:],
                                    rhs=its[dt_][:, base:base + n],
                                    start=(tap == 0), stop=False,
                                )
                        for dt_ in (0, 1):
                            nc.tensor.matmul(
                                out=pss[dt_][:, :n],
                                lhsT=w36[:],
                                rhs=ics[dt_][:, p0 * YP:p0 * YP + n],
                                start=False, stop=True,
                            )
                            nc.scalar.activation(
                                stages[dt_][:, p0 * YP:p0 * YP + n],
                                pss[dt_][:, :n],
                                AF.Identity, bias=bp[:, 0:1], scale=1.0)
                    for dt_ in (0, 1):
                        nc.sync.dma_start(out=predr[t + dt_], in_=stages[dt_][:])

            # ---------------- phase 2: bidirectional scan, exchange every 2
            with (
                tc.tile_pool(name="hfp", bufs=3) as hfp,
                tc.tile_pool(name="hbp", bufs=3) as hbp,
                tc.tile_pool(name="pin", bufs=6) as pinp,
                tc.tile_pool(name="ztp", bufs=4) as ztp,
                tc.tile_pool(name="z2p", bufs=2) as z2p,
                tc.tile_pool(name="mgp", bufs=2) as mgp,
                tc.tile_pool(name="nmp", bufs=2) as nmp,
                tc.tile_pool(name="svp", bufs=2) as svp,
                tc.tile_pool(name="obp", bufs=2) as obp,
            ):
                hf_prev = hb_prev = None

                def scan_step(s, groups, nexts, prevs, pins, n_bnd=2):
                    """emit conv+modrelu for one step (both directions);
                    the last n_bnd groups use the boundary psum pools.
                    Returns (first PE inst per group, last PE inst)."""
                    firsts, last = [], None
                    for gi, (q0, nr) in enumerate(groups):
                        bnd = gi >= len(groups) - n_bnd
                        n = nr * YP
                        # tap-interleave the two directions: consecutive PE
                        # matmuls hit different PSUM banks (drain/fill
                        # overlap) and share each tap's weight load
                        pss = {}
                        for dire in ("f", "b"):
                            pss[dire] = (cpb if bnd else cps).tile(
                                [P, GN], f32, tag="cvx" if bnd
                                else f"cv{dire}", name="cv")
                        grp_first = None
                        if s > 0:
                            for tap in range(9):
                                dxi, dyi = divmod(tap, 3)
                                base = (q0 - 1 + dxi) * YP + dyi
                                for dire in ("f", "b"):
                                    mmi = nc.tensor.matmul(
                                        out=pss[dire][:, :n],
                                        lhsT=wh[tap][:],
                                        rhs=prevs[dire][:, base:base + n],
                                        start=(tap == 0), stop=(tap == 8),
                                    )
                                    if grp_first is None:
                                        grp_first = mmi
                                    last = mmi
                        pb = (q0 - 1) * YP
                        firsts.append(grp_first)
                        for dire in ("f", "b"):
                            h_prev = prevs[dire]
                            pin = pins[dire]
                            h_next = nexts[dire]
                            ps = pss[dire]
                            # evacuate z from PSUM immediately so the
                            # conv bank frees after one DVE copy, not after
                            # the whole modrelu chain (keeps PE run-ahead)
                            zt = ztp.tile([P, GN], f16, tag="zt", name="zt")
                            if s > 0:
                                nc.vector.tensor_tensor(
                                    out=zt[:, :n], in0=ps[:, :n],
                                    in1=pin[:, pb:pb + n], op=ALU.add)
                            else:
                                nc.vector.tensor_copy(
                                    out=zt[:, :n], in_=pin[:, pb:pb + n])
                            zv = zt[:, :n]
                            z2 = z2p.tile([P, GN], f16, tag="z2", name="z2")
                            nc.scalar.activation(z2[:, :n], zv, AF.Square,
                                                 scale=0.015625)
                            mp = (mpb if bnd else mps).tile(
                                [P, GN], f32, tag="mgb" if bnd else "mg",
                                name="mg")
                            nc.tensor.matmul(out=mp[:, :n], lhsT=pm[:],
                                             rhs=z2[:, :n],
                                             start=True, stop=True)
                            rs = mgp.tile([P, GN], f32, tag="rs", name="rs")
                            nc.scalar.activation(rs[:, :n], mp[:, :n],
                                                 AF.Abs_reciprocal_sqrt,
                                                 bias=epst[:, 0:1],
                                                 scale=4096.0)
                            q = nmp.tile([P, GN], f32, tag="q", name="q")
                            nc.vector.tensor_scalar(
                                out=q[:, :n], in0=rs[:, :n],
                                scalar1=bm[:, 0:1], scalar2=1.0,
                                op0=ALU.mult, op1=ALU.add)
                            hb_ = _row(q0)
                            nc.vector.scalar_tensor_tensor(
                                out=h_next[:, hb_:hb_ + n],
                                in0=q[:, :n], scalar=0.0, in1=zv,
                                op0=ALU.max, op1=ALU.mult)
                            if s < NT - 1:
                                vr = h_next[:, hb_:hb_ + n].rearrange(
                                    "p (r y) -> p r y", y=YP)
                                nc.vector.tensor_copy(out=vr[:, :, 0:1],
                                                      in_=vr[:, :, 2:3])
                                nc.vector.tensor_copy(
                                    out=vr[:, :, YP - 1:YP],
                                    in_=vr[:, :, YP - 3:YP - 2])
                    return firsts, last

                def save_combine(s, nexts):
                    def _ivw(h_n):
                        return h_n[:, _row(2):_row(TR - 2)].rearrange(
                            "p (r y) -> p r y", y=YP)[:, :, 1:1 + NY]
                    for dire, t_o in (("f", s), ("b", NT - 1 - s)):
                        h_n = nexts[dire]
                        if s <= 5:
                            nc.sync.dma_start(out=sav[t_o], in_=_ivw(h_n))
                        else:
                            ld = svp.tile([P, ROWS, NY], f16, tag="ld",
                                          name=f"ld{dire}")
                            nc.sync.dma_start(out=ld[:], in_=sav[t_o])
                            ob = obp.tile([P, ROWS, NY], f32, tag="ob",
                                          name=f"ob{dire}")
                            nc.vector.tensor_tensor(
                                out=ob[:], in0=_ivw(h_n),
                                in1=ld[:], op=ALU.add)
                            nc.sync.dma_start(out=out[t_o], in_=ob[:])

                def new_states(s):
                    nexts, pins = {}, {}
                    for dire, hpool in (("f", hfp), ("b", hbp)):
                        t_d = s if dire == "f" else NT - 1 - s
                        pin = pinp.tile([P, PRE_R * YP], f16, tag="pin",
                                        name=f"pin{dire}")
                        nc.sync.dma_start(out=pin[:], in_=predr[t_d])
                        pins[dire] = pin
                        nexts[dire] = hpool.tile([P, FS], f16, tag=f"h{dire}",
                                                 name=f"hn{dire}")
                    return nexts, pins

                # software pipeline across pairs: the AG(k) trigger is
                # followed in PROGRAM ORDER by B_EARLY(k) and A_EARLY(k+1)
                # (~180 matmuls with no halo dependency), so the PE queue
                # never head-of-line blocks on the collective.
                npair = NT // 2
                st_A = [None] * npair
                st_B = [None] * npair

                st_A[0] = new_states(0)
                scan_step(0, A_EARLY, st_A[0][0],
                          {"f": None, "b": None}, st_A[0][1], n_bnd=0)

                for pair in range(npair):
                    sA, sB = 2 * pair, 2 * pair + 1
                    nexts_A, pins_A = st_A[pair]
                    prevs_A = ({"f": None, "b": None} if pair == 0
                               else st_B[pair - 1][0])

                    # A late groups (boundary): need refresh(pair-1)
                    scan_step(sA, A_LATE, nexts_A, prevs_A, pins_A, n_bnd=2)
                    save_combine(sA, nexts_A)

                    # next-step input loads before the collective (pin
                    # DMAs must not queue behind the AllGather)
                    st_B[pair] = new_states(sB)
                    if pair + 1 < npair:
                        st_A[pair + 1] = new_states(sA + 2)

                    # ---- exchange: extend h_A to +-2 halo rows
                    cc_in = dram.tile([NSLOT * P, YP], f16, tag="cci", name="cci")
                    cc_out = dram.tile([NC_CORES * NSLOT * P, YP], f16,
                                       addr_space="Shared", tag="cco", name="cco")
                    cia = cc_in[:]
                    for di, dire in ((0, "f"), (1, "b")):
                        h_n = nexts_A[dire]
                        for si, r0 in ((0, 2), (3, 19)):
                            slot = di * 6 + si
                            o_ap = bass.AP(
                                cia.tensor, int(cia.offset) + slot * P * YP,
                                [[YP, P], [P * YP, 3], [1, YP]])
                            nc.sync.dma_start(
                                out=o_ap,
                                in_=h_n[:, _row(r0):_row(r0 + 3)].rearrange(
                                    "p (r y) -> p r y", y=YP))
                    if os.environ.get("NOAG") == "1":
                        nc.sync.dma_start(out=cc_out[:NSLOT * P, :],
                                          in_=cc_in[:])
                    else:
                        nc.gpsimd.collective_compute(
                            "AllGather", ALU.bypass,
                            replica_groups=[list(range(NC_CORES))],
                            ins=[cc_in[:].opt()], outs=[cc_out[:].opt()],
                        )
                    # B early groups (no halo dependency)
                    nexts_B, pins_B = st_B[pair]
                    scan_step(sB, B_EARLY, nexts_B, nexts_A, pins_B, n_bnd=0)

                    # A early groups of the NEXT pair (no halo dependency)
                    ae_last = None
                    if pair + 1 < npair:
                        _, ae_last = scan_step(sA + 2, A_EARLY,
                                               st_A[pair + 1][0],
                                               nexts_B, st_A[pair + 1][1],
                                               n_bnd=0)

                    # halo gathers (wait on AG)
                    for k, (dire, row) in enumerate(
                            (("f", 0), ("f", 1), ("f", TR - 2), ("f", TR - 1),
                             ("b", 0), ("b", 1), ("b", TR - 2), ("b", TR - 1))):
                        h_n = nexts_A[dire]
                        nc.gpsimd.indirect_dma_start(
                            out=h_n[:, _row(row):_row(row + 1)],
                            out_offset=None,
                            in_=cc_out[:],
                            in_offset=bass.IndirectOffsetOnAxis(
                                ap=hix[:, k:k + 1], axis=0),
                        )

                    # ---- step B late groups (read gathered halo rows);
                    # force them AFTER the next pair's A_EARLY matmuls in the
                    # PE stream so the AllGather hides behind real work
                    scan_step(sB, B_LATE, nexts_B, nexts_A,
                              pins_B, n_bnd=2)
                    save_combine(sB, nexts_B)

                    # ---- local reflect refresh of rows 1 and 22 (edge cores
                    # get reflect copies, interior cores rewrite their own
                    # valid rows) -- no collective
                    if pair < NT // 2 - 1:
                        rf = dram.tile([RSLOT * P, YP], f16, tag="rf", name="rf")
                        rf_v = rf[:].rearrange("(s p) y -> s p y", p=P)
                        for di, dire in ((0, "f"), (1, "b")):
                            h_n = nexts_B[dire]
                            for si, row in enumerate(RF_ROWS):
                                nc.sync.dma_start(
                                    out=rf_v[di * 4 + si],
                                    in_=h_n[:, _row(row):_row(row + 1)])
                        for k, (dire, row) in enumerate(
                                (("f", 1), ("f", TR - 2),
                                 ("b", 1), ("b", TR - 2))):
                            h_n = nexts_B[dire]
                            nc.gpsimd.indirect_dma_start(
                                out=h_n[:, _row(row):_row(row + 1)],
                                out_offset=None,
                                in_=rf[:],
                                in_offset=bass.IndirectOffsetOnAxis(
                                    ap=rix[:, k:k + 1], axis=0),
                            )


    nc.compile()
    _CACHED["nc"] = nc
    return nc


def _complex_lhsT(wr, wi_):
    """[O, I, 3, 3] complex pair -> per-tap lhsT [9, 2*I, 2*O]."""
    O, I = wr.shape[:2]
    lhsT = np.zeros((9, 2 * I, 2 * O), np.float32)
    for tap in range(9):
        kx, ky = divmod(tap, 3)
        lhsT[tap, :I, :O] = wr[:, :, kx, ky].T
        lhsT[tap, I:, :O] = -wi_[:, :, kx, ky].T
        lhsT[tap, :I, O:] = wi_[:, :, kx, ky].T
        lhsT[tap, I:, O:] = wr[:, :, kx, ky].T
    return lhsT


def kernel(**inputs):
    inp_r = np.asarray(inputs["input_r"], np.float32)
    inp_i = np.asarray(inputs["input_i"], np.float32)
    itr_r = np.asarray(inputs["iter_r"], np.float32)
    itr_i = np.asarray(inputs["iter_i"], np.float32)

    # ---- weights ----
    wh2h = _complex_lhsT(np.asarray(inputs["w_h2h_r"]), np.asarray(inputs["w_h2h_i"]))
    wih = _complex_lhsT(np.asarray(inputs["w_ih_r"]), np.asarray(inputs["w_ih_i"]))
    w4 = _complex_lhsT(np.asarray(inputs["w_i2h_r"]), np.asarray(inputs["w_i2h_i"]))
    wi2h = np.ascontiguousarray(w4.reshape(36, P))
    pmat = np.zeros((P, P), np.float32)
    for k in range(P):
        pmat[k, k % HID] = 1.0
        pmat[k, HID + k % HID] = 1.0
    b_pre = np.concatenate([
        inputs["b_i2h_r"] + inputs["b_ih_r"] + inputs["b_h2h_r"],
        inputs["b_i2h_i"] + inputs["b_ih_i"] + inputs["b_h2h_i"],
    ]).astype(np.float32)[:, None]
    b_mod = np.tile(np.asarray(inputs["mod_b"], np.float32), 2)[:, None]

    # ---- activations, reflect-padded by 2: index x+2 <-> global row x ----
    itg = np.concatenate([itr_r[0], itr_i[0]], axis=0).transpose(1, 0, 2, 3)
    itg = np.pad(itg, ((0, 0), (0, 0), (2, 2), (2, 2)), mode="reflect")
    ing = np.concatenate([inp_r[0], inp_i[0]], axis=0).transpose(1, 0, 2, 3)
    ing = np.pad(ing, ((0, 0), (0, 0), (2, 2), (2, 2)), mode="reflect")

    in_maps = []
    for c in range(NC_CORES):
        a = c * ROWS
        # iter tile rows 0..23 <-> global a-2 .. a+21; cols 0..161 <-> y-1..160
        iter_il = np.ascontiguousarray(
            itg[:, :, a:a + TR, 1:1 + YP]).reshape(NT, P, TR * YP)
        # im2col: pre rows p=0..21 <-> global a-1+p; windows of 162 cols;
        # value at (p, j) for tap (dx,dy), ch c4 = input[global a-1+p+dx,
        # padded col j-1+dy] = ing[a-1+p+dx+2, j+dy+1] = ing[a+p+kx, j+ky]
        im2col = np.empty((NT, 36, PRE_R, YP), np.float32)
        for tap in range(9):
            kx, ky = divmod(tap, 3)
            for c4 in range(4):
                im2col[:, tap * 4 + c4] = ing[:, c4, a + kx:a + kx + PRE_R,
                                              ky:ky + YP]
        hidxa = np.zeros((P, 8), np.int32)
        ridxa = np.zeros((P, 4), np.int32)
        pa = np.arange(P)

        def agfl(di, rank, row):
            return (rank * NSLOT + di * 6 + AG_ROWS.index(row)) * P + pa

        for di in (0, 1):
            o = di * 4
            # halo rows 0,1 (global a-2, a-1); TR-2,TR-1 (a+20, a+21)
            if c == 0:
                hidxa[:, o + 0] = agfl(di, 0, 4)       # reflect of global 2
                hidxa[:, o + 1] = agfl(di, 0, 3)       # reflect of global 1
            else:
                hidxa[:, o + 0] = agfl(di, c - 1, 20)  # global a-2
                hidxa[:, o + 1] = agfl(di, c - 1, 21)  # global a-1
            if c == NC_CORES - 1:
                hidxa[:, o + 2] = agfl(di, c, 20)      # reflect of global 158
                hidxa[:, o + 3] = agfl(di, c, 19)      # reflect of global 157
            else:
                hidxa[:, o + 2] = agfl(di, c + 1, 2)   # global a+20
                hidxa[:, o + 3] = agfl(di, c + 1, 3)   # global a+21

        def rfl(di, row):
            return (di * 4 + RF_ROWS.index(row)) * P + pa

        for di in (0, 1):
            o = di * 2
            ridxa[:, o + 0] = rfl(di, 3) if c == 0 else rfl(di, 1)
            ridxa[:, o + 1] = (rfl(di, 20) if c == NC_CORES - 1
                               else rfl(di, 22))
        in_maps.append({
            "iter_il": iter_il.astype(np.float16),
            "im2col": im2col.reshape(NT, 36, PRE_R * YP).astype(np.float16),
            "wh2h": wh2h.astype(np.float16), "wih": wih.astype(np.float16),
            "wi2h": wi2h.astype(np.float16), "pmat": pmat.astype(np.float16),
            "imat": np.eye(P, dtype=np.float16),
            "b_pre": b_pre, "b_mod": b_mod, "hidx": hidxa, "ridx": ridxa,
        })

    nc = _build()
    try:
        res = run_bass_kernel_spmd(nc, in_maps,
                                   core_ids=list(range(NC_CORES)), trace=TRACE)
    except Exception:
        # transient NRT device-state failures recover on retry
        res = run_bass_kernel_spmd(nc, in_maps,
                                   core_ids=list(range(NC_CORES)), trace=TRACE)
    LAST["exec_time_ns"] = res.exec_time_ns
    LAST["results"] = res

    full = np.empty((1, HID, NT, NX, NY, 2), np.float32)
    for c in range(NC_CORES):
        a = c * ROWS
        o = res.results[c]["out"]          # [NT, 128, ROWS, NY]
        full[0, :, :, a:a + ROWS, :, 0] = o[:, :HID].transpose(1, 0, 2, 3)
        full[0, :, :, a:a + ROWS, :, 1] = o[:, HID:].transpose(1, 0, 2, 3)
    return full



# revision 18
# speedup vs baseline: 1.0437x; 1.0437x over previous
"""BCRNN layer (bidirectional convolutional RNN) on 8 Trainium2 NeuronCores.

Problem: nb=1, nc=2, nt=12, nx=160, ny=160, hid=64, K=3, reflect padding,
complex conv decomposed into real convs, modReLU activation, forward +
backward temporal scans, output = sum of the two scans stacked (r, i).

Sharding: spatial rows (nx) split 8 ways (20 rows/core); temporal scans run
locally.  Halo exchange runs every SECOND step (width-2 halos): step A
computes the 20 owned rows, an AllGather + per-core indirect gather then
extends the state to +-2 rows, step B computes 22 rows (one redundant row
each side), and a small local DRAM bounce re-applies the global-edge
reflect rows so no second collective is needed.  All per-core routing
(neighbor vs reflect sources) lives in index input tensors, keeping the
single SPMD program uniform across cores.

Compute per conv: 9 shifted fp16 matmuls (K=128 = stacked real/imag
channels, M=128 = stacked real/imag outputs) accumulated in PSUM, plus one
identity matmul that accumulates pre[t] into the same psum (z never leaves
PSUM).  All matmul rhs operands are single CONTIGUOUS runs over the
padded-row layout (row stride == row length 162); psum rows are 162 wide
and the junk edge columns are overwritten by per-group reflect fixups.
i2h conv is one K=36 matmul over a host-built im2col.  modReLU:
z2 = (z/64)^2 in fp16, one 0/1-matrix matmul pairs partition halves,
rs = AbsRecipSqrt(4096*m2 + 1e-6) = 1/|z|, h = max(0, 1 + b*rs) * z.
"""
import os

import numpy as np

import concourse.bass as bass
import concourse.bacc as bacc
from concourse.bass import _add_dep_helper
import concourse.tile as tile
import concourse.mybir as mybir
from concourse.bass_utils import run_bass_kernel_spmd

P = 128
NC_CORES = 8
NT = 12
NX, NY = 160, 160
HID = 64
ROWS = NX // NC_CORES      # 20 owned rows per core
TR = ROWS + 4              # 24 tile rows (2-row halo each side)
PRE_R = ROWS + 2           # 22 pre rows (one redundant row each side)
YP = NY + 2                # 162 padded cols
FS = TR * YP + 2           # flat h/iter tile size (+1 guard col each side)
NSLOT = 12                 # AG slots: 6 per direction
RSLOT = 8                  # local refresh slots: 4 per direction

f32 = mybir.dt.float32
f16 = mybir.dt.float16
i32 = mybir.dt.int32
AF = mybir.ActivationFunctionType
ALU = mybir.AluOpType

# scan groups as (first output tile row q0, nrows); interior first,
# halo/refresh-dependent groups last
A_EARLY = [(10, 3), (7, 3), (13, 3), (4, 3), (16, 2), (18, 2)]
A_LATE = [(2, 2), (20, 2)]
B_EARLY = [(9, 3), (12, 3), (6, 3), (15, 3), (3, 3), (18, 3)]
B_LATE = [(1, 2), (21, 2)]
# pre groups over the 22 pre rows (pre row p -> iter tile row p+1)
PRE_GROUPS = [(0, 3), (3, 3), (6, 3), (9, 3), (12, 3), (15, 3), (18, 3), (21, 1)]
AG_ROWS = [2, 3, 4, 19, 20, 21]    # contributed h_A tile rows per direction
RF_ROWS = [1, 3, 20, 22]           # contributed h_B tile rows per direction
GN = 3 * YP

_CACHED = {}
TRACE = False
LAST = {}


def _row(q):
    """flat offset of (tile row q, padded col 0) in an FS-sized tile"""
    return 1 + q * YP


def _build():
    if "nc" in _CACHED:
        return _CACHED["nc"]
    nc = bacc.Bacc(None, target_bir_lowering=False, debug=False,
                   num_devices=NC_CORES)

    # ---- I/O ----
    iter_il = nc.dram_tensor("iter_il", [NT, P, TR * YP], f16, kind="ExternalInput")
    im2col = nc.dram_tensor("im2col", [NT, 36, PRE_R * YP], f16, kind="ExternalInput")
    wh2h = nc.dram_tensor("wh2h", [9, P, P], f16, kind="ExternalInput")
    wih = nc.dram_tensor("wih", [9, P, P], f16, kind="ExternalInput")
    wi2h = nc.dram_tensor("wi2h", [36, P], f16, kind="ExternalInput")
    pmat = nc.dram_tensor("pmat", [P, P], f16, kind="ExternalInput")
    imat = nc.dram_tensor("imat", [P, P], f16, kind="ExternalInput")
    b_pre = nc.dram_tensor("b_pre", [P, 1], f32, kind="ExternalInput")
    b_mod = nc.dram_tensor("b_mod", [P, 1], f32, kind="ExternalInput")
    hidx = nc.dram_tensor("hidx", [P, 8], i32, kind="ExternalInput")
    ridx = nc.dram_tensor("ridx", [P, 4], i32, kind="ExternalInput")
    out = nc.dram_tensor("out", [NT, P, ROWS, NY], f32, kind="ExternalOutput")

    # ---- internal DRAM ----
    predr = nc.dram_tensor("predr", [NT, P, PRE_R * YP], f16)
    sav = nc.dram_tensor("sav", [NT, P, ROWS, NY], f16)

    with tile.TileContext(nc) as tc:
        with (
            tc.tile_pool(name="wp", bufs=1) as wp,
            tc.tile_pool(name="dram", bufs=2, space="DRAM") as dram,
            tc.tile_pool(name="cps", bufs=2, space="PSUM") as cps,
            tc.tile_pool(name="cpb", bufs=1, space="PSUM") as cpb,
            tc.tile_pool(name="mps", bufs=2, space="PSUM") as mps,
            tc.tile_pool(name="mpb", bufs=1, space="PSUM") as mpb,
        ):
            # weights / constants
            wh = [wp.tile([P, P], f16, tag=f"wh{k}", name=f"wh{k}") for k in range(9)]
            wi = [wp.tile([P, P], f16, tag=f"wi{k}", name=f"wi{k}") for k in range(9)]
            for k in range(9):
                nc.sync.dma_start(out=wh[k][:], in_=wh2h[k])
                nc.sync.dma_start(out=wi[k][:], in_=wih[k])
            w36 = wp.tile([36, P], f16, tag="w36")
            nc.sync.dma_start(out=w36[:], in_=wi2h[:])
            pm = wp.tile([P, P], f16, tag="pm")
            nc.sync.dma_start(out=pm[:], in_=pmat[:])
            im = wp.tile([P, P], f16, tag="im")
            nc.sync.dma_start(out=im[:], in_=imat[:])
            bp = wp.tile([P, 1], f32, tag="bp")
            nc.sync.dma_start(out=bp[:], in_=b_pre[:])
            bm = wp.tile([P, 1], f32, tag="bm")
            nc.sync.dma_start(out=bm[:], in_=b_mod[:])
            hix = wp.tile([P, 8], i32, tag="hix")
            nc.sync.dma_start(out=hix[:], in_=hidx[:])
            rix = wp.tile([P, 4], i32, tag="rix")
            nc.sync.dma_start(out=rix[:], in_=ridx[:])
            epst = wp.tile([P, 1], f32, tag="epst")
            nc.vector.memset(epst[:], 1e-6)

            # ---------------- phase 1: pre[t] = ih(iter) + i2h(input) + bias
            # pre covers 22 rows (one redundant row each side of owned)
            with (
                tc.tile_pool(name="itp", bufs=2) as itp,
                tc.tile_pool(name="icp", bufs=2) as icp,
                tc.tile_pool(name="prs", bufs=2) as prs,
            ):
                for t in range(NT):
                    it = itp.tile([P, FS], f16, tag="it")
                    nc.sync.dma_start(out=it[:, 1:1 + TR * YP], in_=iter_il[t])
                    ic = icp.tile([36, PRE_R * YP], f16, tag="ic")
                    nc.sync.dma_start(out=ic[:], in_=im2col[t])
                    stage = prs.tile([P, PRE_R * YP], f16, tag="stage")
                    for p0, nr in PRE_GROUPS:
                        n = nr * YP
                        ps = cps.tile([P, GN], f32, tag="cvf")
                        for tap in range(9):
                            dxi, dyi = divmod(tap, 3)
                            base = (p0 + dxi) * YP + dyi
                            nc.tensor.matmul(
                                out=ps[:, :n],
                                lhsT=wi[tap][:],
                                rhs=it[:, base:base + n],
                                start=(tap == 0), stop=False,
                            )
                        nc.tensor.matmul(
                            out=ps[:, :n],
                            lhsT=w36[:],
                            rhs=ic[:, p0 * YP:p0 * YP + n],
                            start=False, stop=True,
                        )
                        nc.scalar.activation(stage[:, p0 * YP:p0 * YP + n],
                                             ps[:, :n],
                                             AF.Identity, bias=bp[:, 0:1], scale=1.0)
                    nc.sync.dma_start(out=predr[t], in_=stage[:])

            # ---------------- phase 2: bidirectional scan, exchange every 2
            with (
                tc.tile_pool(name="hfp", bufs=3) as hfp,
                tc.tile_pool(name="hbp", bufs=3) as hbp,
                tc.tile_pool(name="pin", bufs=6) as pinp,
                tc.tile_pool(name="ztp", bufs=4) as ztp,
                tc.tile_pool(name="z2p", bufs=2) as z2p,
                tc.tile_pool(name="mgp", bufs=2) as mgp,
                tc.tile_pool(name="nmp", bufs=2) as nmp,
                tc.tile_pool(name="svp", bufs=2) as svp,
                tc.tile_pool(name="obp", bufs=2) as obp,
            ):
                hf_prev = hb_prev = None

                def scan_step(s, groups, nexts, prevs, pins, n_bnd=2):
                    """emit conv+modrelu for one step (both directions);
                    the last n_bnd groups use the boundary psum pools.
                    Returns (first PE inst per group, last PE inst)."""
                    firsts, last = [], None
                    for gi, (q0, nr) in enumerate(groups):
                        bnd = gi >= len(groups) - n_bnd
                        n = nr * YP
                        # tap-interleave the two directions: consecutive PE
                        # matmuls hit different PSUM banks (drain/fill
                        # overlap) and share each tap's weight load
                        pss = {}
                        for dire in ("f", "b"):
                            pss[dire] = (cpb if bnd else cps).tile(
                                [P, GN], f32, tag="cvx" if bnd
                                else f"cv{dire}", name="cv")
                        grp_first = None
                        if s > 0:
                            for tap in range(9):
                                dxi, dyi = divmod(tap, 3)
                                base = (q0 - 1 + dxi) * YP + dyi
                                for dire in ("f", "b"):
                                    mmi = nc.tensor.matmul(
                                        out=pss[dire][:, :n],
                                        lhsT=wh[tap][:],
                                        rhs=prevs[dire][:, base:base + n],
                                        start=(tap == 0), stop=(tap == 8),
                                    )
                                    if grp_first is None:
                                        grp_first = mmi
                                    last = mmi
                        pb = (q0 - 1) * YP
                        firsts.append(grp_first)
                        for dire in ("f", "b"):
                            h_prev = prevs[dire]
                            pin = pins[dire]
                            h_next = nexts[dire]
                            ps = pss[dire]
                            # evacuate z from PSUM immediately so the
                            # conv bank frees after one DVE copy, not after
                            # the whole modrelu chain (keeps PE run-ahead)
                            zt = ztp.tile([P, GN], f16, tag="zt", name="zt")
                            if s > 0:
                                nc.vector.tensor_tensor(
                                    out=zt[:, :n], in0=ps[:, :n],
                                    in1=pin[:, pb:pb + n], op=ALU.add)
                            else:
                                nc.vector.tensor_copy(
                                    out=zt[:, :n], in_=pin[:, pb:pb + n])
                            zv = zt[:, :n]
                            z2 = z2p.tile([P, GN], f16, tag="z2", name="z2")
                            nc.scalar.activation(z2[:, :n], zv, AF.Square,
                                                 scale=0.015625)
                            mp = (mpb if bnd else mps).tile(
                                [P, GN], f32, tag="mgb" if bnd else "mg",
                                name="mg")
                            nc.tensor.matmul(out=mp[:, :n], lhsT=pm[:],
                                             rhs=z2[:, :n],
                                             start=True, stop=True)
                            rs = mgp.tile([P, GN], f32, tag="rs", name="rs")
                            nc.scalar.activation(rs[:, :n], mp[:, :n],
                                                 AF.Abs_reciprocal_sqrt,
                                                 bias=epst[:, 0:1],
                                                 scale=4096.0)
                            q = nmp.tile([P, GN], f32, tag="q", name="q")
                            nc.vector.tensor_scalar(
                                out=q[:, :n], in0=rs[:, :n],
                                scalar1=bm[:, 0:1], scalar2=1.0,
                                op0=ALU.mult, op1=ALU.add)
                            hb_ = _row(q0)
                            nc.vector.scalar_tensor_tensor(
                                out=h_next[:, hb_:hb_ + n],
                                in0=q[:, :n], scalar=0.0, in1=zv,
                                op0=ALU.max, op1=ALU.mult)
                            if s < NT - 1:
                                vr = h_next[:, hb_:hb_ + n].rearrange(
                                    "p (r y) -> p r y", y=YP)
                                nc.vector.tensor_copy(out=vr[:, :, 0:1],
                                                      in_=vr[:, :, 2:3])
                                nc.vector.tensor_copy(
                                    out=vr[:, :, YP - 1:YP],
                                    in_=vr[:, :, YP - 3:YP - 2])
                    return firsts, last

                def save_combine(s, nexts):
                    def _ivw(h_n):
                        return h_n[:, _row(2):_row(TR - 2)].rearrange(
                            "p (r y) -> p r y", y=YP)[:, :, 1:1 + NY]
                    for dire, t_o in (("f", s), ("b", NT - 1 - s)):
                        h_n = nexts[dire]
                        if s <= 5:
                            nc.sync.dma_start(out=sav[t_o], in_=_ivw(h_n))
                        else:
                            ld = svp.tile([P, ROWS, NY], f16, tag="ld",
                                          name=f"ld{dire}")
                            nc.sync.dma_start(out=ld[:], in_=sav[t_o])
                            ob = obp.tile([P, ROWS, NY], f32, tag="ob",
                                          name=f"ob{dire}")
                            nc.vector.tensor_tensor(
                                out=ob[:], in0=_ivw(h_n),
                                in1=ld[:], op=ALU.add)
                            nc.sync.dma_start(out=out[t_o], in_=ob[:])

                def new_states(s):
                    nexts, pins = {}, {}
                    for dire, hpool in (("f", hfp), ("b", hbp)):
                        t_d = s if dire == "f" else NT - 1 - s
                        pin = pinp.tile([P, PRE_R * YP], f16, tag="pin",
                                        name=f"pin{dire}")
                        nc.sync.dma_start(out=pin[:], in_=predr[t_d])
                        pins[dire] = pin
                        nexts[dire] = hpool.tile([P, FS], f16, tag=f"h{dire}",
                                                 name=f"hn{dire}")
                    return nexts, pins

                # software pipeline across pairs: the AG(k) trigger is
                # followed in PROGRAM ORDER by B_EARLY(k) and A_EARLY(k+1)
                # (~180 matmuls with no halo dependency), so the PE queue
                # never head-of-line blocks on the collective.
                npair = NT // 2
                st_A = [None] * npair
                st_B = [None] * npair

                st_A[0] = new_states(0)
                scan_step(0, A_EARLY, st_A[0][0],
                          {"f": None, "b": None}, st_A[0][1], n_bnd=0)

                for pair in range(npair):
                    sA, sB = 2 * pair, 2 * pair + 1
                    nexts_A, pins_A = st_A[pair]
                    prevs_A = ({"f": None, "b": None} if pair == 0
                               else st_B[pair - 1][0])

                    # A late groups (boundary): need refresh(pair-1)
                    scan_step(sA, A_LATE, nexts_A, prevs_A, pins_A, n_bnd=2)
                    save_combine(sA, nexts_A)

                    # next-step input loads before the collective (pin
                    # DMAs must not queue behind the AllGather)
                    st_B[pair] = new_states(sB)
                    if pair + 1 < npair:
                        st_A[pair + 1] = new_states(sA + 2)

                    # ---- exchange: extend h_A to +-2 halo rows
                    cc_in = dram.tile([NSLOT * P, YP], f16, tag="cci", name="cci")
                    cc_out = dram.tile([NC_CORES * NSLOT * P, YP], f16,
                                       addr_space="Shared", tag="cco", name="cco")
                    cia = cc_in[:]
                    for di, dire in ((0, "f"), (1, "b")):
                        h_n = nexts_A[dire]
                        for si, r0 in ((0, 2), (3, 19)):
                            slot = di * 6 + si
                            o_ap = bass.AP(
                                cia.tensor, int(cia.offset) + slot * P * YP,
                                [[YP, P], [P * YP, 3], [1, YP]])
                            nc.sync.dma_start(
                                out=o_ap,
                                in_=h_n[:, _row(r0):_row(r0 + 3)].rearrange(
                                    "p (r y) -> p r y", y=YP))
                    if os.environ.get("NOAG") == "1":
                        nc.sync.dma_start(out=cc_out[:NSLOT * P, :],
                                          in_=cc_in[:])
                    else:
                        nc.gpsimd.collective_compute(
                            "AllGather", ALU.bypass,
                            replica_groups=[list(range(NC_CORES))],
                            ins=[cc_in[:].opt()], outs=[cc_out[:].opt()],
                        )
                    # B early groups (no halo dependency)
                    nexts_B, pins_B = st_B[pair]
                    scan_step(sB, B_EARLY, nexts_B, nexts_A, pins_B, n_bnd=0)

                    # A early groups of the NEXT pair (no halo dependency)
                    ae_last = None
                    if pair + 1 < npair:
                        _, ae_last = scan_step(sA + 2, A_EARLY,
                                               st_A[pair + 1][0],
                                               nexts_B, st_A[pair + 1][1],
                                               n_bnd=0)

                    # halo gathers (wait on AG)
                    for k, (dire, row) in enumerate(
                            (("f", 0), ("f", 1), ("f", TR - 2), ("f", TR - 1),
                             ("b", 0), ("b", 1), ("b", TR - 2), ("b", TR - 1))):
                        h_n = nexts_A[dire]
                        nc.gpsimd.indirect_dma_start(
                            out=h_n[:, _row(row):_row(row + 1)],
                            out_offset=None,
                            in_=cc_out[:],
                            in_offset=bass.IndirectOffsetOnAxis(
                                ap=hix[:, k:k + 1], axis=0),
                        )

                    # ---- step B late groups (read gathered halo rows);
                    # force them AFTER the next pair's A_EARLY matmuls in the
                    # PE stream so the AllGather hides behind real work
                    scan_step(sB, B_LATE, nexts_B, nexts_A,
                              pins_B, n_bnd=2)
                    save_combine(sB, nexts_B)

                    # ---- local reflect refresh of rows 1 and 22 (edge cores
                    # get reflect copies, interior cores rewrite their own
                    # valid rows) -- no collective
                    if pair < NT // 2 - 1:
                        rf = dram.tile([RSLOT * P, YP], f16, tag="rf", name="rf")
                        rf_v = rf[:].rearrange("(s p) y -> s p y", p=P)
                        for di, dire in ((0, "f"), (1, "b")):
                            h_n = nexts_B[dire]
                            for si, row in enumerate(RF_ROWS):
                                nc.sync.dma_start(
                                    out=rf_v[di * 4 + si],
                                    in_=h_n[:, _row(row):_row(row + 1)])
                        for k, (dire, row) in enumerate(
                                (("f", 1), ("f", TR - 2),
                                 ("b", 1), ("b", TR - 2))):
                            h_n = nexts_B[dire]
                            nc.gpsimd.indirect_dma_start(
                                out=h_n[:, _row(row):_row(row + 1)],
                                out_offset=None,
                                in_=rf[:],
                                in_offset=bass.IndirectOffsetOnAxis(
                                    ap=rix[:, k:k + 1], axis=0),
                            )


    nc.compile()
    _CACHED["nc"] = nc
    return nc


def _complex_lhsT(wr, wi_):
    """[O, I, 3, 3] complex pair -> per-tap lhsT [9, 2*I, 2*O]."""
    O, I = wr.shape[:2]
    lhsT = np.zeros((9, 2 * I, 2 * O), np.float32)
    for tap in range(9):
        kx, ky = divmod(tap, 3)
        lhsT[tap, :I, :O] = wr[:, :, kx, ky].T
        lhsT[tap, I:, :O] = -wi_[:, :, kx, ky].T
        lhsT[tap, :I, O:] = wi_[:, :, kx, ky].T
        lhsT[tap, I:, O:] = wr[:, :, kx, ky].T
    return lhsT


def kernel(**inputs):
    inp_r = np.asarray(inputs["input_r"], np.float32)
    inp_i = np.asarray(inputs["input_i"], np.float32)
    itr_r = np.asarray(inputs["iter_r"], np.float32)
    itr_i = np.asarray(inputs["iter_i"], np.float32)

    # ---- weights ----
    wh2h = _complex_lhsT(np.asarray(inputs["w_h2h_r"]), np.asarray(inputs["w_h2h_i"]))
    wih = _complex_lhsT(np.asarray(inputs["w_ih_r"]), np.asarray(inputs["w_ih_i"]))
    w4 = _complex_lhsT(np.asarray(inputs["w_i2h_r"]), np.asarray(inputs["w_i2h_i"]))
    wi2h = np.ascontiguousarray(w4.reshape(36, P))
    pmat = np.zeros((P, P), np.float32)
    for k in range(P):
        pmat[k, k % HID] = 1.0
        pmat[k, HID + k % HID] = 1.0
    b_pre = np.concatenate([
        inputs["b_i2h_r"] + inputs["b_ih_r"] + inputs["b_h2h_r"],
        inputs["b_i2h_i"] + inputs["b_ih_i"] + inputs["b_h2h_i"],
    ]).astype(np.float32)[:, None]
    b_mod = np.tile(np.asarray(inputs["mod_b"], np.float32), 2)[:, None]

    # ---- activations, reflect-padded by 2: index x+2 <-> global row x ----
    itg = np.concatenate([itr_r[0], itr_i[0]], axis=0).transpose(1, 0, 2, 3)
    itg = np.pad(itg, ((0, 0), (0, 0), (2, 2), (2, 2)), mode="reflect")
    ing = np.concatenate([inp_r[0], inp_i[0]], axis=0).transpose(1, 0, 2, 3)
    ing = np.pad(ing, ((0, 0), (0, 0), (2, 2), (2, 2)), mode="reflect")

    in_maps = []
    for c in range(NC_CORES):
        a = c * ROWS
        # iter tile rows 0..23 <-> global a-2 .. a+21; cols 0..161 <-> y-1..160
        iter_il = np.ascontiguousarray(
            itg[:, :, a:a + TR, 1:1 + YP]).reshape(NT, P, TR * YP)
        # im2col: pre rows p=0..21 <-> global a-1+p; windows of 162 cols;
        # value at (p, j) for tap (dx,dy), ch c4 = input[global a-1+p+dx,
        # padded col j-1+dy] = ing[a-1+p+dx+2, j+dy+1] = ing[a+p+kx, j+ky]
        im2col = np.empty((NT, 36, PRE_R, YP), np.float32)
        for tap in range(9):
            kx, ky = divmod(tap, 3)
            for c4 in range(4):
                im2col[:, tap * 4 + c4] = ing[:, c4, a + kx:a + kx + PRE_R,
                                              ky:ky + YP]
        hidxa = np.zeros((P, 8), np.int32)
        ridxa = np.zeros((P, 4), np.int32)
        pa = np.arange(P)

        def agfl(di, rank, row):
            return (rank * NSLOT + di * 6 + AG_ROWS.index(row)) * P + pa

        for di in (0, 1):
            o = di * 4
            # halo rows 0,1 (global a-2, a-1); TR-2,TR-1 (a+20, a+21)
            if c == 0:
                hidxa[:, o + 0] = agfl(di, 0, 4)       # reflect of global 2
                hidxa[:, o + 1] = agfl(di, 0, 3)       # reflect of global 1
            else:
                hidxa[:, o + 0] = agfl(di, c - 1, 20)  # global a-2
                hidxa[:, o + 1] = agfl(di, c - 1, 21)  # global a-1
            if c == NC_CORES - 1:
                hidxa[:, o + 2] = agfl(di, c, 20)      # reflect of global 158
                hidxa[:, o + 3] = agfl(di, c, 19)      # reflect of global 157
            else:
                hidxa[:, o + 2] = agfl(di, c + 1, 2)   # global a+20
                hidxa[:, o + 3] = agfl(di, c + 1, 3)   # global a+21

        def rfl(di, row):
            return (di * 4 + RF_ROWS.index(row)) * P + pa

        for di in (0, 1):
            o = di * 2
            ridxa[:, o + 0] = rfl(di, 3) if c == 0 else rfl(di, 1)
            ridxa[:, o + 1] = (rfl(di, 20) if c == NC_CORES - 1
                               else rfl(di, 22))
        in_maps.append({
            "iter_il": iter_il.astype(np.float16),
            "im2col": im2col.reshape(NT, 36, PRE_R * YP).astype(np.float16),
            "wh2h": wh2h.astype(np.float16), "wih": wih.astype(np.float16),
            "wi2h": wi2h.astype(np.float16), "pmat": pmat.astype(np.float16),
            "imat": np.eye(P, dtype=np.float16),
            "b_pre": b_pre, "b_mod": b_mod, "hidx": hidxa, "ridx": ridxa,
        })

    nc = _build()
    try:
        res = run_bass_kernel_spmd(nc, in_maps,
                                   core_ids=list(range(NC_CORES)), trace=TRACE)
    except Exception:
        # transient NRT device-state failures recover on retry
        res = run_bass_kernel_spmd(nc, in_maps,
                                   core_ids=list(range(NC_CORES)), trace=TRACE)
    LAST["exec_time_ns"] = res.exec_time_ns
    LAST["results"] = res

    full = np.empty((1, HID, NT, NX, NY, 2), np.float32)
    for c in range(NC_CORES):
        a = c * ROWS
        o = res.results[c]["out"]          # [NT, 128, ROWS, NY]
        full[0, :, :, a:a + ROWS, :, 0] = o[:, :HID].transpose(1, 0, 2, 3)
        full[0, :, :, a:a + ROWS, :, 1] = o[:, HID:].transpose(1, 0, 2, 3)
    return full

